# revision 28
# baseline (speedup 1.0000x reference)
"""Multi-head attention (B=2, S=2048, D=768, H=12) on 8 Trainium2 NeuronCores.

Sharding: core c handles batch b = c//4 and heads [3*(c%4), 3*(c%4)+3).
Each core computes its 3 heads' attention weights [3, S, S] plus a partial
(pre-bias) output projection [S, D]; the host sums the 4 head-group partials
per batch and adds the bias.

Device algorithm per core (causal; upper-triangle blocks never computed —
the output buffer is pre-zeroed by the runtime):
  1. QT/KT = (W q_b^T) in [dk, s] layout, V in natural [s, dk] layout, all
     from bf16 inputs (host-converted), stored per-512-column tile so later
     stages can start as soon as the first block is projected.
  2. Pass 1 per (i-tile, head): scores = QT_h^T KT_h over columns
     [0, (it+1)*128) in <=1024-wide PSUM rows, 128x128 tri mask on the
     diagonal corner, exp (scale 1/8) with fused row-sum accumulation,
     normalize by the reciprocal row sum, DMA the attention weights out.
  3. Pass 2: scores recomputed transposed [j, i] (cheaper than on-chip
     transposition), exp'd, consumed by the AV matmul into unnormalized
     U^T [64, S] per head (PSUM accumulation over j-tiles).
  4. Per i-tile and head: U_h^T^T @ Wo_h^T -> PSUM [128, 768], scaled by the
     per-row reciprocal and accumulated across heads on VectorE; interleaved
     with pass 2 so the tail overlaps.
"""

import math

import numpy as np

B, S, D, H = 2, 2048, 768, 12
DK = D // H            # 64
HC = 3                 # heads per core
G = HC * DK            # 192
NCORES = 8
SCALE = math.sqrt(DK)  # 8.0
P = 128
NB_D = D // P          # 6
SBLK = 512
N_SBLK = S // SBLK     # 4
N_ITILE = S // P       # 16
MASKVAL = -1e9

_cached = {}


def _build_nc():
    import concourse.mybir as mybir
    import concourse.tile as tile
    from concourse import bacc
    from concourse.bass import ts
    from contextlib import ExitStack

    FP32 = mybir.dt.float32
    FP32R = mybir.dt.float32r
    BF16 = mybir.dt.bfloat16
    EXPF = mybir.ActivationFunctionType.Exp
    ADD = mybir.AluOpType.add
    MULT = mybir.AluOpType.mult
    IS_GE = mybir.AluOpType.is_ge

    nc = bacc.Bacc("TRN2", target_bir_lowering=False, debug=False, num_devices=NCORES)

    xq = nc.dram_tensor("xqT", [D, S], BF16, kind="ExternalInput").ap()
    xk = nc.dram_tensor("xkT", [D, S], BF16, kind="ExternalInput").ap()
    xv = nc.dram_tensor("xvT", [D, S], BF16, kind="ExternalInput").ap()
    wq = nc.dram_tensor("wqT", [D, G], BF16, kind="ExternalInput").ap()
    wk = nc.dram_tensor("wkT", [D, G], BF16, kind="ExternalInput").ap()
    wv = nc.dram_tensor("wvT", [D, 256], BF16, kind="ExternalInput").ap()
    wo = nc.dram_tensor("woT", [G, D], FP32R, kind="ExternalInput").ap()
    p_out = nc.dram_tensor("p_out", [HC, S, S], FP32, kind="ExternalOutput").ap()
    o_out = nc.dram_tensor("o_out", [S, D], FP32, kind="ExternalOutput").ap()

    def r(ap):
        return ap.bitcast(FP32R) if ap.dtype == FP32 else ap

    with ExitStack() as ctx:
        tc = ctx.enter_context(tile.TileContext(nc))
        const_pool = ctx.enter_context(tc.tile_pool(name="const", bufs=1))
        w_pool = ctx.enter_context(tc.tile_pool(name="w", bufs=1))
        x_pool = ctx.enter_context(tc.tile_pool(name="x", bufs=6))
        qkv_pool = ctx.enter_context(tc.tile_pool(name="qkv", bufs=1))
        row_pool = ctx.enter_context(tc.tile_pool(name="rows", bufs=5))
        small_pool = ctx.enter_context(tc.tile_pool(name="small", bufs=4))
        st_pool = ctx.enter_context(tc.tile_pool(name="st", bufs=6))
        ut_pool = ctx.enter_context(tc.tile_pool(name="ut", bufs=1))
        oacc_pool = ctx.enter_context(tc.tile_pool(name="oacc", bufs=3))
        ps_row = ctx.enter_context(tc.tile_pool(name="ps_row", bufs=3, space="PSUM"))
        ps_b = ctx.enter_context(tc.tile_pool(name="ps_b", bufs=2, space="PSUM"))

        # --- constant causal masks (128x128 additive tri) ---------------
        tri1 = const_pool.tile([P, P], FP32, tag="tri1")  # 0 where col<=row
        nc.gpsimd.memset(tri1[:], 0.0)
        nc.gpsimd.affine_select(
            out=tri1[:], in_=tri1[:], compare_op=IS_GE, fill=MASKVAL,
            base=0, pattern=[[-1, P]], channel_multiplier=1,
        )
        tri2 = const_pool.tile([P, P], FP32, tag="tri2")  # 0 where col>=row
        nc.gpsimd.memset(tri2[:], 0.0)
        nc.gpsimd.affine_select(
            out=tri2[:], in_=tri2[:], compare_op=IS_GE, fill=MASKVAL,
            base=0, pattern=[[1, P]], channel_multiplier=-1,
        )

        # --- weights ----------------------------------------------------
        wq_sb = w_pool.tile([P, NB_D, G], BF16, tag="wq")
        nc.sync.dma_start(wq_sb[:], wq.rearrange("(o p) g -> p o g", p=P))
        wk_sb = w_pool.tile([P, NB_D, G], BF16, tag="wk")
        nc.sync.dma_start(wk_sb[:], wk.rearrange("(o p) g -> p o g", p=P))
        wv_sb = w_pool.tile([P, NB_D, 256], BF16, tag="wv")
        nc.sync.dma_start(wv_sb[:], wv.rearrange("(o p) g -> p o g", p=P))
        wo_sb = w_pool.tile([DK, HC, D], FP32R, tag="wo")
        nc.sync.dma_start(wo_sb[:], wo.rearrange("(h p) d -> p h d", p=DK))

        # --- phase 1: projections (per 512-col block tiles) -------------
        qta = [qkv_pool.tile([P, SBLK], FP32R, tag=f"qta{i}", name=f"qta{i}")
               for i in range(N_SBLK)]
        qtb = [qkv_pool.tile([DK, SBLK], FP32R, tag=f"qtb{i}", name=f"qtb{i}")
               for i in range(N_SBLK)]
        kta = [qkv_pool.tile([P, SBLK], FP32R, tag=f"kta{i}", name=f"kta{i}")
               for i in range(N_SBLK)]
        ktb = [qkv_pool.tile([DK, SBLK], FP32R, tag=f"ktb{i}", name=f"ktb{i}")
               for i in range(N_SBLK)]
        vt = [qkv_pool.tile([P, N_SBLK, G], FP32R, tag=f"v{i}", name=f"v{i}")
              for i in range(N_SBLK)]

        xre_q = xq.rearrange("(o p) s -> p o s", p=P)
        xre_k = xk.rearrange("(o p) s -> p o s", p=P)
        xre_v = xv.rearrange("(o p) s -> p o s", p=P)

        def emit_proj(sb):
            copy_eng = nc.scalar.copy if sb < 2 else nc.vector.tensor_copy
            for which, xre, w_sb, out_a, out_b in (
                (0, xre_q, wq_sb, qta[sb], qtb[sb]),
                (1, xre_k, wk_sb, kta[sb], ktb[sb]),
                (2, xre_v, wv_sb, None, None),
            ):
                x_sb = x_pool.tile([P, NB_D, SBLK], BF16, tag="xT", name="x_sb")
                nc.sync.dma_start(x_sb[:], xre[:, :, ts(sb, SBLK)])
                if which < 2:
                    pq = ps_row.tile([P, 2 * SBLK], FP32, tag="r",
                                     name="pq")[:, 0:SBLK]
                    for d in range(NB_D):
                        nc.tensor.matmul(
                            pq[:], r(w_sb[:, d, 0:P]), r(x_sb[:, d, :]),
                            start=(d == 0), stop=(d == NB_D - 1),
                        )
                    copy_eng(out_a[:], pq[:])
                    pqb = ps_b.tile([DK, SBLK], FP32, tag="b", name="pqb")
                    for d in range(NB_D):
                        nc.tensor.matmul(
                            pqb[:], r(w_sb[:, d, P:G]), r(x_sb[:, d, :]),
                            start=(d == 0), stop=(d == NB_D - 1),
                        )
                    copy_eng(out_b[:], pqb[:])
                else:
                    for sti in range(SBLK // P):
                        pv = ps_row.tile([P, 2 * SBLK], FP32, tag="r",
                                         name="pv")[:, 0:SBLK]
                        for d in range(NB_D):
                            nc.tensor.matmul(
                                pv[:, 0:256], r(x_sb[:, d, ts(sti, P)]),
                                r(wv_sb[:, d, :]),
                                start=(d == 0), stop=(d == NB_D - 1),
                            )
                        copy_eng(vt[sb][:, sti, :], pv[:, 0:G])

        def q_ap(h, c0, w):
            sb, rel = c0 // SBLK, c0 % SBLK
            assert (c0 + w - 1) // SBLK == sb
            if h < 2:
                return qta[sb][h * DK:(h + 1) * DK, rel:rel + w]
            return qtb[sb][:, rel:rel + w]

        def k_ap(h, c0, w):
            sb, rel = c0 // SBLK, c0 % SBLK
            assert (c0 + w - 1) // SBLK == sb
            if h < 2:
                return kta[sb][h * DK:(h + 1) * DK, rel:rel + w]
            return ktb[sb][:, rel:rel + w]

        def v_ap(h, jt):
            return vt[jt // 4][:, jt % 4, h * DK:(h + 1) * DK]

        # per-(head, i-tile) reciprocal row sums, reused in the Wo stage
        recip_all = const_pool.tile([P, HC, N_ITILE], FP32, tag="recip")

        # --- phase 2: scores pass 1 + softmax + P output ----------------
        def emit_p1(it):
            ncols = (it + 1) * P
            nchunk = (ncols + 1023) // 1024
            for h in range(HC):
                erow = row_pool.tile([P, S], FP32, tag="erow", name="erow")
                sums = small_pool.tile([P, 2], FP32, tag="sums", name="sums")
                for ch in range(nchunk):
                    c0 = ch * 1024
                    w = min(ncols - c0, 1024)
                    psr = ps_row.tile([P, 2 * SBLK], FP32, tag="r", name="psr")
                    for q in range((w + SBLK - 1) // SBLK):
                        nw = min(SBLK, w - q * SBLK)
                        nc.tensor.matmul(
                            psr[:, q * SBLK:q * SBLK + nw],
                            r(q_ap(h, it * P, P)),
                            r(k_ap(h, c0 + q * SBLK, nw)),
                            start=True, stop=True,
                        )
                    if ch == nchunk - 1:
                        nc.vector.tensor_tensor(
                            psr[:, w - P:w], psr[:, w - P:w], tri1[:], ADD
                        )
                    nc.scalar.activation(
                        erow[:, c0:c0 + w], psr[:, 0:w], EXPF,
                        scale=1.0 / SCALE, accum_out=sums[:, ch:ch + 1],
                    )
                rslot = recip_all[:, h, it:it + 1]
                if nchunk == 1:
                    nc.vector.reciprocal(rslot, sums[:, 0:1])
                else:
                    rtmp = small_pool.tile([P, 1], FP32, tag="rtmp", name="rtmp")
                    nc.vector.tensor_tensor(
                        rtmp[:], sums[:, 0:1], sums[:, 1:2], ADD
                    )
                    nc.vector.reciprocal(rslot, rtmp[:])
                nc.vector.tensor_scalar_mul(
                    erow[:, 0:ncols], erow[:, 0:ncols], rslot
                )
                nc.sync.dma_start(p_out[h, ts(it, P), 0:ncols], erow[:, 0:ncols])

        # --- phase 3: scores pass 2 (transposed) + AV, Wo interleaved ---
        ut = [ut_pool.tile([DK, S], FP32R, tag=f"ut{h}", name=f"ut{h}")
              for h in range(HC)]

        def wo_tile(it, act_h0=False):
            oacc = oacc_pool.tile([P, D], FP32, tag="oacc", name="oacc")
            for h in range(HC):
                po = ps_row.tile([P, 2 * SBLK], FP32, tag="r", name="po")
                nc.tensor.matmul(
                    po[:, 0:SBLK], r(ut[h][:, ts(it, P)]),
                    r(wo_sb[:, h, 0:SBLK]), start=True, stop=True,
                )
                nc.tensor.matmul(
                    po[:, SBLK:D], r(ut[h][:, ts(it, P)]),
                    r(wo_sb[:, h, SBLK:D]), start=True, stop=True,
                )
                rc = recip_all[:, h, it:it + 1]
                if h == 0:
                    if act_h0:
                        nc.scalar.mul(oacc[:], po[:, 0:D], rc)
                    else:
                        nc.vector.tensor_scalar_mul(oacc[:], po[:, 0:D], rc)
                else:
                    nc.vector.scalar_tensor_tensor(
                        oacc[:], po[:, 0:D], rc, oacc[:], MULT, ADD
                    )
            nc.sync.dma_start(o_out[ts(it, P), :], oacc[:])

        def emit_p2_seg(ihalf, h, pa, jts):
            if True:
                for jt in jts:
                    delta = jt * P - ihalf * 1024
                    ps2 = ps_row.tile([P, 2 * SBLK], FP32, tag="r", name="ps2")
                    lo = max(delta, 0)
                    for hf in (0, 1):
                        c0 = max(delta - hf * SBLK, 0)   # first live rel col
                        if c0 >= SBLK:
                            continue
                        nc.tensor.matmul(
                            ps2[:, hf * SBLK + c0:(hf + 1) * SBLK],
                            r(k_ap(h, jt * P, P)),
                            r(q_ap(h, ihalf * 1024 + hf * SBLK + c0,
                                   SBLK - c0)),
                            start=True, stop=True,
                        )
                        doff = delta - hf * SBLK
                        if 0 <= doff < SBLK:
                            nc.vector.tensor_tensor(
                                ps2[:, hf * SBLK + doff:hf * SBLK + doff + P],
                                ps2[:, hf * SBLK + doff:hf * SBLK + doff + P],
                                tri2[:], ADD,
                            )
                    est = st_pool.tile([P, 2 * SBLK], FP32R, tag="st",
                                       name="est")
                    nc.scalar.activation(
                        est[:, lo:1024], ps2[:, lo:1024], EXPF, scale=1.0 / SCALE
                    )
                    for hf in (0, 1):
                        c0 = max(delta - hf * SBLK, 0)
                        if c0 >= SBLK:
                            continue
                        nc.tensor.matmul(
                            pa[hf][:, c0:SBLK],
                            r(v_ap(h, jt)),
                            r(est[:, hf * SBLK + c0:(hf + 1) * SBLK]),
                            start=(jt == 0),
                            stop=(jt == ihalf * 8 + 4 * (hf + 1) - 1),
                        )
        def emit_p2_start():
            return [ps_b.tile([DK, SBLK], FP32, tag="b", name=f"pa{i}")
                    for i in range(2)]

        def emit_p2_end(ihalf, h, pa):
            for hf in range(2):
                nc.vector.tensor_copy(
                    ut[h][:, ihalf * 1024 + hf * SBLK:
                           ihalf * 1024 + (hf + 1) * SBLK],
                    pa[hf][:],
                )
        # --- explicit interleaved schedule ------------------------------
        def p2_chunk(ihalf, h, between):
            """Emit pass-2 for (ihalf, h) in 4-jt segments, interleaving the
            thunks in `between` after successive segments."""
            njt = 8 * (ihalf + 1)
            pa = emit_p2_start()
            segs = [range(s, min(s + 2, njt)) for s in range(0, njt, 2)]
            bi = iter(between)
            for k, seg in enumerate(segs):
                emit_p2_seg(ihalf, h, pa, seg)
                for thunk in next(bi, ()):
                    thunk()
            emit_p2_end(ihalf, h, pa)
            for rest in bi:
                for thunk in rest:
                    thunk()

        P1 = lambda it: (lambda: emit_p1(it))
        WO = lambda it: (lambda: wo_tile(it))

        emit_proj(0)
        emit_p1(0)
        emit_proj(1)
        emit_p1(1)
        emit_p1(2)
        emit_proj(2)
        emit_p1(3)
        emit_proj(3)
        p2_chunk(0, 0, [[P1(4)], [P1(5)]])
        p2_chunk(0, 1, [[P1(6)], [P1(7)]])
        p2_chunk(0, 2, [[P1(8)], [P1(9)]])
        p2_chunk(1, 0, [[WO(0), P1(10)], [WO(1)], [P1(11), WO(2)], [WO(3)]])
        p2_chunk(1, 1, [[P1(12), WO(4)], [WO(5)], [P1(13), WO(6)], [WO(7)]])
        p2_chunk(1, 2, [[P1(14)], [], [], []])
        wo_tile(8, act_h0=True)
        wo_tile(9, act_h0=True)
        emit_p1(15)
        for it in range(10, 16):
            wo_tile(it, act_h0=True)

    nc.finalize()
    return nc


def _get_nc():
    if "nc" not in _cached:
        _cached["nc"] = _build_nc()
    return _cached["nc"]


def _make_in_maps(query, key, value, Wq, Wk, Wv, Wo):
    import ml_dtypes

    bf16 = ml_dtypes.bfloat16
    in_maps = []
    wvT_pad = np.zeros((D, 256), dtype=bf16)
    for c in range(NCORES):
        b = c // 4
        h0 = (c % 4) * HC
        rows = slice(h0 * DK, (h0 + HC) * DK)
        wvT_c = wvT_pad.copy()
        wvT_c[:, :G] = Wv[rows].T.astype(bf16)
        in_maps.append({
            "xqT": np.ascontiguousarray(query[b].T.astype(bf16)),
            "xkT": np.ascontiguousarray(key[b].T.astype(bf16)),
            "xvT": np.ascontiguousarray(value[b].T.astype(bf16)),
            "wqT": np.ascontiguousarray(Wq[rows].T.astype(bf16)),
            "wkT": np.ascontiguousarray(Wk[rows].T.astype(bf16)),
            "wvT": wvT_c,
            "woT": np.ascontiguousarray(Wo[:, rows].T),
        })
    return in_maps


def _run_device(in_maps, trace=False):
    from concourse.bass_utils import run_bass_kernel_spmd

    nc = _get_nc()
    return run_bass_kernel_spmd(nc, in_maps, core_ids=list(range(NCORES)),
                                trace=trace)


def _assemble(results, bo):
    P_full = np.zeros((B, H, S, S), dtype=np.float32)
    O_full = np.zeros((B, S, D), dtype=np.float32)
    for c in range(NCORES):
        b = c // 4
        h0 = (c % 4) * HC
        P_full[b, h0:h0 + HC] = results[c]["p_out"]
        O_full[b] += results[c]["o_out"]
    O_full += bo[None, None, :].astype(np.float32)
    return O_full, P_full


def _numpy_fallback(query, key, value, mask, Wq, Wk, Wv, Wo, bo):
    q = np.asarray(query, np.float32)
    k = np.asarray(key, np.float32)
    v = np.asarray(value, np.float32)
    Q = (q @ Wq.T).reshape(B, S, H, DK)
    K = (k @ Wk.T).reshape(B, S, H, DK)
    V = (v @ Wv.T).reshape(B, S, H, DK)
    scores = np.einsum("bqhd,bkhd->bhqk", Q, K).astype(np.float32) / SCALE
    scores = np.where(mask[:, None, :, :] == 0, np.float32(-1e9), scores)
    scores -= scores.max(axis=-1, keepdims=True)
    np.exp(scores, out=scores)
    scores /= scores.sum(axis=-1, keepdims=True)
    out = np.einsum("bhqk,bkhd->bqhd", scores, V).reshape(B, S, D)
    return (out @ Wo.T + bo).astype(np.float32), scores


def kernel(query, key, value, mask, Wq, Wk, Wv, Wo, bo):
    query = np.asarray(query, np.float32)
    key = np.asarray(key, np.float32)
    value = np.asarray(value, np.float32)
    mask = np.asarray(mask)
    Wq = np.asarray(Wq, np.float32)
    Wk = np.asarray(Wk, np.float32)
    Wv = np.asarray(Wv, np.float32)
    Wo = np.asarray(Wo, np.float32)
    bo = np.asarray(bo, np.float32)

    tril = np.tril(np.ones((S, S), dtype=mask.dtype))
    if not all(np.array_equal(mask[b], tril) for b in range(B)):
        return _numpy_fallback(query, key, value, mask, Wq, Wk, Wv, Wo, bo)

    in_maps = _make_in_maps(query, key, value, Wq, Wk, Wv, Wo)
    res = _run_device(in_maps)
    return _assemble(res.results, bo)
